# revision 56
# baseline (speedup 1.0000x reference)
"""AssignmentSimilarityNet GNN message-passing kernel for 8 Trainium2
NeuronCores.

Sharding: track (A) dimension split across 8 cores (32 tracks each).
Edge tensors, track embeds and messages-to-A stay local; messages-to-B
(sum over A) are all-reduced each step; MLP weights replicated.

Layouts (per core, feature-major: features on SBUF partitions):
  - edge/pair tensors: (128 feat, 8192 pairs), pairs a-major
    (pair = a*256 + b); compute in 512-pair groups (2 tracks) so every
    f32r matmul streams the max 512 columns per weight load.
  - the track-embed term of the edge MLP is constant along b, so it is
    folded into the h1-relu bias (beta) instead of a 512-cycle matmul.
  - msg_a rides on the u-relu: even groups via scalar-engine accum_out,
    odd groups via DVE tensor_reduce in phase C (DVE tensor_scalar's
    accum_out silently drops the max stage - do not use it).
  - msg_b lives entirely on the Pool(Q7) queue, one accumulator per
    group-half, pre-folded one group early; the last group's u-relu
    (on the fast Act queue) gates only two Q7 adds before the AR
    doorbell.  Each half gets its own AllReduce (AR is linear), so the
    first hides under the rest of the sweep.  AR results return via
    sync-DMA + DVE rounding copy so the Pool queue never blocks.
  - logits: c2 uses wc2 replicated into an 8-wide stationary; the copy
    takes PSUM row 0 into a (1, 8192) row, streamed out in 4 quarter
    DMAs (contiguous, no scatter).
  - per-step schedule: edge MLP sweep -> AR triggers -> classifier +
    track update + next-step group opens (AR cover) -> AR consume
    (current-embed update). Step 7 skips messages/AR/node updates
    entirely (dead code in the reference).
"""
import numpy as np

A = 256          # tracks
B = 256          # current detections
AL = A // 8      # tracks per core (32)
REID = 512
D = 128          # ND == ED
STEPS = 8
NP = AL * B      # pairs per core (8192)
CH = B           # half-group = one track row (256 pairs)
NCH = NP // CH   # 32
GCH = 2 * CH     # pair group = two track rows (512 pairs)
NG = NP // GCH   # 16
PREG = 4         # groups pre-opened for the next step during AR cover

_BUILD_CACHE = {}


def _build():
    if "nc" in _BUILD_CACHE:
        return _BUILD_CACHE["nc"]
    import concourse.bacc as bacc
    import concourse.mybir as mybir
    import concourse.tile as tile

    F32 = mybir.dt.float32
    F32R = mybir.dt.float32r
    F16 = mybir.dt.float16
    AF = mybir.ActivationFunctionType
    ALU = mybir.AluOpType

    nc = bacc.Bacc(None, target_bir_lowering=False)

    def din(name, shape):
        return nc.dram_tensor(name, shape, F32, kind="ExternalInput")

    tfT = din("tfT", [REID, AL])
    trkf = din("trkf", [AL, REID])
    cfT = din("cfT", [REID, B])
    curf = din("curf", [B, REID])
    trkg = din("trkg", [AL, 5])
    curg = din("curg", [B, 5])
    wlinT = din("wlinT", [REID, D])
    wein1T = din("wein1T", [6, D])
    wein2T = din("wein2T", [D, D])
    we1T = din("we1T", [4 * D, D])
    we2T = din("we2T", [D, D])
    wn1T = din("wn1T", [2 * D, D])
    wn2T = din("wn2T", [D, D])
    wc1T = din("wc1T", [D, D])
    wc2p8 = din("wc2p8", [D, 8])
    ballT = din("ballT", [D, 9])
    out = nc.dram_tensor("out", [STEPS, NP], F32, kind="ExternalOutput")

    with tile.TileContext(nc) as tc:
        with (
            tc.tile_pool(name="const", bufs=1) as cp,
            tc.tile_pool(name="state", bufs=1) as st,
            tc.tile_pool(name="work", bufs=1) as wk,
            tc.tile_pool(name="p1", bufs=5, space="PSUM") as pp1,
            tc.tile_pool(name="pmm", bufs=2, space="PSUM") as pmm,
            tc.tile_pool(name="plg", bufs=1, space="PSUM") as plgp,
            tc.tile_pool(name="dram", bufs=1, space="DRAM") as dr,
        ):
            # ---------------- feature loads ----------------
            tf_t = st.tile([128, 4 * AL], F32R)       # 4 K-tiles of (128, 32)
            cf_t = st.tile([128, 4 * B], F32R)        # 4 K-tiles of (128, 256)
            nc.gpsimd.dma_start(
                tf_t[:, :].rearrange("p (j c) -> p j c", j=4),
                tfT.rearrange("(j p) c -> p j c", p=128))
            nc.gpsimd.dma_start(
                cf_t[:, :].rearrange("p (j c) -> p j c", j=4),
                cfT.rearrange("(j p) c -> p j c", p=128))
            trkf_t = wk.tile([AL, REID], F32)
            nc.sync.dma_start(trkf_t[:], trkf[:])
            curf_t0 = wk.tile([128, REID], F32)
            curf_t1 = wk.tile([128, REID], F32)
            nc.sync.dma_start(curf_t0[:], curf[0:128, :])
            nc.sync.dma_start(curf_t1[:], curf[128:256, :])
            trkg_t = wk.tile([AL, 5], F32)
            nc.sync.dma_start(trkg_t[:], trkg[:])
            curg_t0 = wk.tile([128, 5], F32)
            curg_t1 = wk.tile([128, 5], F32)
            nc.sync.dma_start(curg_t0[:], curg[0:128, :])
            nc.sync.dma_start(curg_t1[:], curg[128:256, :])

            # ---------------- weight / bias loads ----------------
            we1_t = cp.tile([128, 4 * D], F32R)
            wlin_t = cp.tile([128, 4 * D], F32R)
            wn1_t = cp.tile([128, 2 * D], F32R)
            nc.gpsimd.dma_start(
                we1_t[:, :].rearrange("p (j c) -> p j c", j=4),
                we1T.rearrange("(j p) c -> p j c", p=128))
            nc.gpsimd.dma_start(
                wlin_t[:, :].rearrange("p (j c) -> p j c", j=4),
                wlinT.rearrange("(j p) c -> p j c", p=128))
            nc.gpsimd.dma_start(
                wn1_t[:, :].rearrange("p (j c) -> p j c", j=2),
                wn1T.rearrange("(j p) c -> p j c", p=128))
            wein1_t = cp.tile([6, D], F32R)
            wein2_t = cp.tile([128, D], F32R)
            we2_t = cp.tile([128, D], F32R)
            wn2_t = cp.tile([128, D], F32R)
            wc1_t = cp.tile([128, D], F32R)
            wc2p8_t = cp.tile([128, 8], F32R)
            for dst, src in [(wein1_t, wein1T), (wein2_t, wein2T),
                             (we2_t, we2T), (wn2_t, wn2T), (wc1_t, wc1T),
                             (wc2p8_t, wc2p8)]:
                nc.gpsimd.dma_start(dst[:], src[:])
            bnames = ["blin", "bein1", "bein2", "be1", "be2", "bn1",
                      "bn2", "bc1", "bc2"]
            ball_t = cp.tile([128, 9], F32)
            nc.sync.dma_start(ball_t[:], ballT[:, :])
            biases = {nm: ball_t[:, i:i + 1] for i, nm in enumerate(bnames)}

            # ---------------- reid norms ----------------
            sq_t = wk.tile([AL, REID], F32, tag="sq", bufs=2)
            nc.vector.tensor_mul(sq_t[:], trkf_t[:], trkf_t[:])
            sst = wk.tile([AL, 1], F32)
            nc.vector.tensor_reduce(sst[:], sq_t[:], mybir.AxisListType.X, ALU.add)
            rt = wk.tile([AL, 1], F32)
            nc.vector.reciprocal(rt[:], sst[:])
            inv_t = wk.tile([AL, 1], F32)
            nc.scalar.activation(inv_t[:], rt[:], AF.Sqrt)

            invc = []
            for i, ct in enumerate((curf_t0, curf_t1)):
                sq_c = wk.tile([128, REID], F32, name=f"sq_c{i}", tag="sq", bufs=2)
                nc.vector.tensor_mul(sq_c[:], ct[:], ct[:])
                ssc = wk.tile([128, 1], F32, name=f"ssc{i}")
                nc.vector.tensor_reduce(ssc[:], sq_c[:], mybir.AxisListType.X,
                                        ALU.add)
                rc = wk.tile([128, 1], F32, name=f"rc{i}")
                nc.vector.reciprocal(rc[:], ssc[:])
                ic = wk.tile([128, 1], F32, name=f"ic{i}")
                nc.scalar.activation(ic[:], rc[:], AF.Sqrt)
                invc.append(ic)

            # ---------------- current-side geometry -> bcast rows ----------
            # rows of cstage: 0 xb, 1 yb, 2 hb, 3 ln hb, 4 ln wb, 5 tb, 6 invc
            # rows 0-5 (pure geometry) are staged and broadcast separately
            # from row 6 (invc, which waits on the reid-norm chain), so the
            # f0-f4 feature pipeline starts ~15us earlier.
            cstage = dr.tile([7, B], F32)
            for i, gt in enumerate((curg_t0, curg_t1)):
                half = slice(128 * i, 128 * (i + 1))
                cg = wk.tile([128, 6], F32, name=f"cg{i}")
                nc.vector.tensor_add(cg[:, 0:1], gt[:, 0:1], gt[:, 2:3])
                nc.vector.tensor_scalar_mul(cg[:, 0:1], cg[:, 0:1], 0.5)
                nc.vector.tensor_add(cg[:, 1:2], gt[:, 1:2], gt[:, 3:4])
                nc.vector.tensor_scalar_mul(cg[:, 1:2], cg[:, 1:2], 0.5)
                nc.vector.tensor_sub(cg[:, 2:3], gt[:, 3:4], gt[:, 1:2])
                wb = wk.tile([128, 1], F32, name=f"wb{i}")
                nc.vector.tensor_sub(wb[:], gt[:, 2:3], gt[:, 0:1])
                nc.scalar.activation(cg[:, 3:4], cg[:, 2:3], AF.Ln)
                nc.scalar.activation(cg[:, 4:5], wb[:], AF.Ln)
                nc.vector.tensor_copy(cg[:, 5:6], gt[:, 4:5])
                nc.scalar.dma_start(cstage[0:6, half].transpose((1, 0)),
                                    cg[:])
                nc.scalar.dma_start(cstage[6:7, half].transpose((1, 0)),
                                    invc[i][:])
            bcall = wk.tile([AL, 7 * B], F32)
            nc.scalar.dma_start(
                bcall[:, 0:6 * B], cstage[0:6, :].partition_broadcast(AL)
                .rearrange("p r b -> p (r b)"))
            nc.scalar.dma_start(
                bcall[:, 6 * B:7 * B],
                cstage[6:7, :].partition_broadcast(AL)
                .rearrange("p r b -> p (r b)"))
            bc = {nm: bcall[:, B * r:B * (r + 1)]
                  for r, nm in enumerate(["xb", "yb", "hb", "lnhb",
                                          "lnwb", "tb", "invc"])}

            # ---------------- track-side geometry scalars ----------------
            xt = wk.tile([AL, 1], F32)
            nc.vector.tensor_add(xt[:], trkg_t[:, 0:1], trkg_t[:, 2:3])
            nc.vector.tensor_scalar_mul(xt[:], xt[:], 0.5)
            yt = wk.tile([AL, 1], F32)
            nc.vector.tensor_add(yt[:], trkg_t[:, 1:2], trkg_t[:, 3:4])
            nc.vector.tensor_scalar_mul(yt[:], yt[:], 0.5)
            ht = wk.tile([AL, 1], F32)
            nc.vector.tensor_sub(ht[:], trkg_t[:, 3:4], trkg_t[:, 1:2])
            wt = wk.tile([AL, 1], F32)
            nc.vector.tensor_sub(wt[:], trkg_t[:, 2:3], trkg_t[:, 0:1])
            lnht = wk.tile([AL, 1], F32)
            nc.scalar.activation(lnht[:], ht[:], AF.Ln)
            lnwt = wk.tile([AL, 1], F32)
            nc.scalar.activation(lnwt[:], wt[:], AF.Ln)

            # ---------------- edge features (AL, B) each ----------------
            den = wk.tile([AL, B], F32)
            nc.vector.tensor_scalar_add(den[:], bc["hb"][:], ht[:, 0:1])
            rden = wk.tile([AL, B], F32)
            nc.vector.reciprocal(rden[:], den[:])

            fall = wk.tile([AL, 6 * B], F32R, name="f_all")
            feats = []
            f0 = fall[:, 0 * B:1 * B]
            nc.vector.tensor_scalar(f0, bc["xb"][:], xt[:, 0:1], 2.0,
                                    ALU.subtract, ALU.mult)
            nc.vector.tensor_mul(f0, f0, rden[:])
            feats.append(f0)
            f1 = fall[:, 1 * B:2 * B]
            nc.vector.tensor_scalar(f1, bc["yb"][:], yt[:, 0:1], 2.0,
                                    ALU.subtract, ALU.mult)
            nc.vector.tensor_mul(f1, f1, rden[:])
            feats.append(f1)
            f2 = fall[:, 2 * B:3 * B]
            nc.vector.tensor_scalar(f2, bc["lnwb"][:], -1.0, lnwt[:, 0:1],
                                    ALU.mult, ALU.add)
            feats.append(f2)
            f3 = fall[:, 3 * B:4 * B]
            nc.vector.tensor_scalar(f3, bc["lnhb"][:], -1.0, lnht[:, 0:1],
                                    ALU.mult, ALU.add)
            feats.append(f3)
            f4 = fall[:, 4 * B:5 * B]
            nc.vector.tensor_scalar_sub(f4, bc["tb"][:], trkg_t[:, 4:5])
            feats.append(f4)

            # dist_reid = 1 - gram * inv_t * inv_c
            pg = pmm.tile([AL, B], F32, tag="pmm")
            for j in range(4):
                nc.tensor.matmul(pg[:], tf_t[:, AL * j:AL * (j + 1)],
                                 cf_t[:, B * j:B * (j + 1)],
                                 start=(j == 0), stop=(j == 3))
            f5 = fall[:, 5 * B:6 * B]
            nc.vector.tensor_scalar(f5, pg[:], inv_t[:, 0:1], None,
                                    ALU.mult)
            nc.vector.tensor_mul(f5, f5, bc["invc"][:])
            nc.vector.tensor_scalar(f5, f5, -1.0, 1.0, ALU.mult, ALU.add)
            feats.append(f5)

            # ---------------- transpose features -> efT (6, 8192) ----------
            ef_stage = dr.tile([6, NP], F32R)
            nc.sync.dma_start(
                ef_stage[:, :].rearrange("f (a b) -> a f b", a=AL),
                fall[:].rearrange("a (f b) -> a f b", f=6))
            upds = [st.tile([128, NP], F32R, name="updA"),
                    st.tile([128, NP], F32R, name="updB")]
            efT_t = upds[0][0:6, :]
            nc.sync.dma_start(efT_t, ef_stage[:])

            # ---------------- initial node embeds ----------------
            pt = pmm.tile([128, AL], F32, tag="pmm")
            for j in range(4):
                nc.tensor.matmul(pt[:], wlin_t[:, 128 * j:128 * (j + 1)],
                                 tf_t[:, AL * j:AL * (j + 1)],
                                 start=(j == 0), stop=(j == 3))
            te = [st.tile([128, AL], F32R, name="teA"),
                  st.tile([128, AL], F32R, name="teB")]
            nc.scalar.activation(te[0][:], pt[:], AF.Relu,
                                 bias=biases["blin"][:, 0:1])
            pc_ = pmm.tile([128, B], F32, tag="pmm")
            for j in range(4):
                nc.tensor.matmul(pc_[:], wlin_t[:, 128 * j:128 * (j + 1)],
                                 cf_t[:, B * j:B * (j + 1)],
                                 start=(j == 0), stop=(j == 3))
            ce = [st.tile([128, B], F32R, name="ceA"),
                  st.tile([128, B], F32R, name="ceB")]
            nc.scalar.activation(ce[0][:], pc_[:], AF.Relu,
                                 bias=biases["blin"][:, 0:1])

            # ---------------- fixed_edge = mlp2(edge_feats) ----------------
            fixedT = st.tile([128, NP], F32R)
            ein_pend = None
            for g in range(NG):
                sl = slice(GCH * g, GCH * (g + 1))
                p1 = pp1.tile([128, GCH], F32, tag="p1")
                nc.tensor.matmul(p1[:], wein1_t[:], efT_t[:, sl],
                                 start=True, stop=True)
                h = wk.tile([128, GCH], F32R, tag="h1", bufs=4)
                if g % 2 == 0:
                    nc.scalar.activation(h[:], p1[:], AF.Relu,
                                         bias=biases["bein1"][:, 0:1])
                else:
                    nc.vector.tensor_scalar(h[:], p1[:],
                                            biases["bein1"][:, 0:1], 0.0,
                                            ALU.add, ALU.max)
                if ein_pend is not None:
                    pg_, ph = ein_pend
                    slp = slice(GCH * pg_, GCH * (pg_ + 1))
                    p2 = pmm.tile([128, GCH], F32, tag="pmm")
                    nc.tensor.matmul(p2[:], wein2_t[:], ph[:],
                                     start=True, stop=True)
                    if pg_ % 2 == 0:
                        nc.vector.tensor_scalar(fixedT[:, slp], p2[:],
                                                biases["bein2"][:, 0:1], 0.0,
                                                ALU.add, ALU.max)
                    else:
                        nc.scalar.activation(fixedT[:, slp], p2[:], AF.Relu,
                                             bias=biases["bein2"][:, 0:1])
                ein_pend = (g, h)
            pg_, ph = ein_pend
            slp = slice(GCH * pg_, GCH * (pg_ + 1))
            p2 = pmm.tile([128, GCH], F32, tag="pmm")
            nc.tensor.matmul(p2[:], wein2_t[:], ph[:], start=True, stop=True)
            nc.scalar.activation(fixedT[:, slp], p2[:], AF.Relu,
                                 bias=biases["bein2"][:, 0:1])

            # ---------------- message-passing steps ----------------
            def open_group(g, k, u_prev):
                """fixed/upd contributions of the edge-MLP hidden layer.
                The track-embed term is constant along b, so it rides in the
                h1 relu bias (beta) instead of a 512-cycle matmul."""
                sl = slice(GCH * g, GCH * (g + 1))
                p1 = pp1.tile([128, GCH], F32, tag="p1")
                if k == 0:
                    nc.tensor.matmul(p1[:], wefu_t[:], fixedT[:, sl],
                                     start=True, stop=False)
                else:
                    nc.tensor.matmul(p1[:], we1_t[:, 384:512], fixedT[:, sl],
                                     start=True, stop=False)
                    nc.tensor.matmul(p1[:], we1_t[:, 256:384], u_prev[:, sl],
                                     start=False, stop=False)
                return p1

            def make_beta(te_t, kk):
                # beta[:, a] = we1a @ te[:, a] + be1  (per-track h1 bias)
                ptc = pmm.tile([128, AL], F32, tag="pmm")
                nc.tensor.matmul(ptc[:], we1_t[:, 0:128], te_t[:],
                                 start=True, stop=True)
                bt = wk.tile([128, AL], F32, name=f"beta{kk}",
                             tag="beta", bufs=2)
                nc.vector.tensor_scalar_add(bt[:], ptc[:],
                                            biases["be1"][:, 0:1])
                return bt

            # step-0 edge MLP reads fixedT for both the upd and fixed slots
            wefu_t = cp.tile([128, D], F32R)
            nc.vector.tensor_add(wefu_t[:], we1_t[:, 256:384],
                                 we1_t[:, 384:512])

            preopened = {}
            beta = make_beta(te[0], 0)
            for g in range(PREG):
                preopened[g] = open_group(g, 0, fixedT)

            for k in range(STEPS):
                u_prev = fixedT if k == 0 else upds[(k + 1) % 2]
                u_cur = upds[k % 2]
                te_cur, te_nxt = te[k % 2], te[(k + 1) % 2]
                ce_cur, ce_nxt = ce[k % 2], ce[(k + 1) % 2]
                last = (k == STEPS - 1)

                msga = wk.tile([128, AL], F32, tag="msga", bufs=2)
                # msg_b accumulators: two halves (groups 0-7 / 8-15), each
                # split DVE/GpSimd.  AllReduce is linear, so each half gets
                # its own AR: the first is triggered mid-sweep and hides
                # fully under the rest of phase A.
                macc = {}
                mb_outs = []

                def macc_tile(key):
                    if key not in macc:
                        macc[key] = wk.tile([128, GCH], F32,
                                            name=f"macc_{key}_{k}",
                                            tag=f"macc{key}", bufs=1)
                        return macc[key], True
                    return macc[key], False

                prefold = {}

                def make_prefold(half):
                    # the accumulator is complete one group before the half
                    # ends: collapse its two a-parity halves early so the
                    # last group's u relu gates only two Q7 adds before the
                    # AR doorbell.  The whole msg_b path lives on the Pool
                    # queue, which tracks phase A closely (DVE/Act are the
                    # backlogged queues).
                    pf = wk.tile([128, B], F32, name=f"pf{half}_{k}",
                                 tag="pfold", bufs=2)
                    qa, _ = macc_tile(f"q{half}")
                    nc.gpsimd.tensor_add(pf[:], qa[:, 0:B], qa[:, B:GCH])
                    prefold[half] = pf

                def trigger_ar(half, u_f):
                    pf = prefold.pop(half)
                    msgb = wk.tile([128, B], F32, tag="msgb", bufs=2)
                    nc.gpsimd.tensor_add(msgb[:], pf[:], u_f[:, 0:B])
                    nc.gpsimd.tensor_add(msgb[:], msgb[:], u_f[:, B:GCH])
                    mb_in = dr.tile([128, B], F32, tag="mbin", bufs=4)
                    mb_out = dr.tile([128, B], F32, tag="mbout", bufs=4,
                                     addr_space="Shared")
                    nc.sync.dma_start(mb_in[:], msgb[:])
                    nc.gpsimd.collective_compute(
                        "AllReduce", mybir.AluOpType.add,
                        replica_groups=[list(range(8))],
                        ins=[mb_in.opt()], outs=[mb_out.opt()])
                    mb_outs.append(mb_out)

                # ---- phase A: edge MLP sweep (16 groups of 2 tracks) ----
                def tail_a(g):
                    sl = slice(GCH * g, GCH * (g + 1))
                    h = h1s.pop(g)
                    p2 = pmm.tile([128, GCH], F32, tag="pmm")
                    nc.tensor.matmul(p2[:], we2_t[:], h[:],
                                     start=True, stop=True)
                    if last:
                        nc.scalar.activation(u_cur[:, sl], p2[:], AF.Relu,
                                             bias=biases["be2"][:, 0:1])
                        return
                    half = g // (NG // 2)
                    pos = g % (NG // 2)
                    if g % 2 == 0 or pos == NG // 2 - 1:
                        # Act relu halves; accum_out gives msg_a for free.
                        # Also used for the last group of each half so the
                        # AR trigger chain rides the fast Act queue.
                        for hh in range(2):
                            hsl = slice(GCH * g + CH * hh,
                                        GCH * g + CH * (hh + 1))
                            nc.scalar.activation(
                                u_cur[:, hsl], p2[:, CH * hh:CH * (hh + 1)],
                                AF.Relu, bias=biases["be2"][:, 0:1],
                                accum_out=msga[:, 2 * g + hh:2 * g + hh + 1])
                    else:
                        # plain DVE relu; msg_a for these groups comes from
                        # tensor_reduce during phase C
                        nc.vector.tensor_scalar(u_cur[:, sl], p2[:],
                                                biases["be2"][:, 0:1], 0.0,
                                                ALU.add, ALU.max)
                    u_f = u_cur[:, sl].bitcast(F32)
                    if pos == NG // 2 - 1:
                        # last group of the half feeds the AR input directly
                        trigger_ar(half, u_f)
                        return
                    acc, first = macc_tile(f"q{half}")
                    if first:
                        nc.gpsimd.tensor_copy(acc[:], u_f)
                    else:
                        nc.gpsimd.tensor_add(acc[:], acc[:], u_f)
                    if pos == NG // 2 - 2:
                        make_prefold(half)

                h1s = {}
                for g in range(NG):
                    if g in preopened:
                        p1 = preopened.pop(g)
                    else:
                        p1 = open_group(g, k, u_prev)
                    nc.tensor.matmul(
                        p1[:], we1_t[:, 128:256],
                        ce_cur[:, :].to_broadcast((128, B, 2))
                        .transpose((0, 2, 1)), start=False, stop=True)
                    h = wk.tile([128, GCH], F32R, tag="h1", bufs=4)
                    for hh in range(2):
                        a = 2 * g + hh
                        csl = slice(CH * hh, CH * (hh + 1))
                        if g % 2 == 1:
                            nc.scalar.activation(h[:, csl], p1[:, csl],
                                                 AF.Relu,
                                                 bias=beta[:, a:a + 1])
                        else:
                            nc.vector.tensor_scalar(h[:, csl], p1[:, csl],
                                                    beta[:, a:a + 1], 0.0,
                                                    ALU.add, ALU.max)
                    h1s[g] = h
                    if g > 2:
                        tail_a(g - 3)
                tail_a(NG - 3)
                tail_a(NG - 2)
                tail_a(NG - 1)

                # ---- phase C: classifier for this step (AR cover) ----
                # c2 with wc2 replicated into an 8-wide stationary: every
                # PSUM row 0..7 gets the group's logits; the copy takes row
                # g%8, so lg_sb rows interleave groups and the out DMA is 8
                # partitions x 4KB, all f32r precision, no transpose.
                lg_sb = wk.tile([1, NP], F32, tag="lgs", bufs=1)
                hcs = {}

                def c2_mm(g):
                    hc = hcs.pop(g)
                    pl2 = plgp.tile([8, GCH], F32, tag="plg")
                    nc.tensor.matmul(pl2[:], wc2p8_t[:], hc[:],
                                     start=True, stop=True)
                    dst = lg_sb[0:1, GCH * g:GCH * (g + 1)]
                    nc.scalar.activation(dst, pl2[0:1, :], AF.Identity,
                                         bias=biases["bc2"][0:1, 0:1])
                    if g % 4 == 3:   # stream the finished quarter out
                        q4 = g // 4
                        nc.sync.dma_start(
                            out[k:k + 1, 2048 * q4:2048 * (q4 + 1)],
                            lg_sb[0:1, 2048 * q4:2048 * (q4 + 1)])

                for g in range(NG):
                    sl = slice(GCH * g, GCH * (g + 1))
                    p3 = pmm.tile([128, GCH], F32, tag="pmm")
                    nc.tensor.matmul(p3[:], wc1_t[:], u_cur[:, sl],
                                     start=True, stop=True)
                    hc = wk.tile([128, GCH], F32R, tag="hc", bufs=3)
                    nc.vector.tensor_scalar(hc[:], p3[:],
                                            biases["bc1"][:, 0:1], 0.0,
                                            ALU.add, ALU.max)
                    hcs[g] = hc
                    if g > 1:
                        c2_mm(g - 2)
                c2_mm(NG - 2)
                c2_mm(NG - 1)

                if last:
                    break

                # ---- track-side node update (local msg_a) ----
                for g in (1, 3, 5, 9, 11, 13):   # odd groups' msg_a via reduce
                    sl = slice(GCH * g, GCH * (g + 1))
                    nc.vector.tensor_reduce(
                        msga[:, 2 * g:2 * g + 2],
                        u_cur[:, sl].bitcast(F32)
                        .rearrange("p (a b) -> p a b", a=2),
                        mybir.AxisListType.X, ALU.add)
                msga_r = wk.tile([128, AL], F32R, tag="msgar", bufs=2)
                nc.vector.tensor_copy(msga_r[:], msga[:])
                pt1 = pmm.tile([128, AL], F32, tag="pmm")
                nc.tensor.matmul(pt1[:], wn1_t[:, 0:128], te_cur[:],
                                 start=True, stop=False)
                nc.tensor.matmul(pt1[:], wn1_t[:, 128:256],
                                 msga_r[:], start=False, stop=True)
                tn1 = wk.tile([128, AL], F32R, tag="tn1", bufs=2)
                nc.vector.tensor_scalar(tn1[:], pt1[:], biases["bn1"][:, 0:1],
                                        0.0, ALU.add, ALU.max)
                pt2 = pmm.tile([128, AL], F32, tag="pmm")
                nc.tensor.matmul(pt2[:], wn2_t[:], tn1[:], start=True,
                                 stop=True)
                nc.vector.tensor_scalar(te_nxt[:], pt2[:], biases["bn2"][:, 0:1],
                                        0.0, ALU.add, ALU.max)

                # ---- pre-open next step's first groups (more AR cover) ----
                beta = make_beta(te_nxt, k + 1)
                for g in range(PREG):
                    preopened[g] = open_group(g, k + 1, u_cur)

                # ---- phase D: consume ARs -> current-embed update ----
                # AR results land via sync-engine DMAs, then a DVE copy
                # rounds f32 -> f32r for the matmul.
                msgb_fs = []
                for hf, mo in enumerate(mb_outs):
                    ms = wk.tile([128, B], F32, name=f"mbs{hf}_{k}",
                                 tag="msgbs", bufs=4)
                    nc.sync.dma_start(ms[:], mo[:])
                    mf = wk.tile([128, B], F32R, name=f"mbf{hf}_{k}",
                                 tag="msgbf", bufs=4)
                    nc.vector.tensor_copy(mf[:], ms[:])
                    msgb_fs.append(mf)
                pc1 = pmm.tile([128, B], F32, tag="pmm")
                nc.tensor.matmul(pc1[:], wn1_t[:, 0:128], ce_cur[:],
                                 start=True, stop=False)
                nc.tensor.matmul(pc1[:], wn1_t[:, 128:256], msgb_fs[0][:],
                                 start=False, stop=False)
                nc.tensor.matmul(pc1[:], wn1_t[:, 128:256], msgb_fs[1][:],
                                 start=False, stop=True)
                cn1 = wk.tile([128, B], F32R, tag="cn1", bufs=2)
                nc.scalar.activation(cn1[:], pc1[:], AF.Relu,
                                     bias=biases["bn1"][:, 0:1])
                pc2 = pmm.tile([128, B], F32, tag="pmm")
                nc.tensor.matmul(pc2[:], wn2_t[:], cn1[:], start=True,
                                 stop=True)
                nc.scalar.activation(ce_nxt[:], pc2[:], AF.Relu,
                                     bias=biases["bn2"][:, 0:1])

    nc.finalize()
    _BUILD_CACHE["nc"] = nc
    return nc


def _make_in_maps(inputs):
    f32 = np.float32

    def c(x):
        return np.ascontiguousarray(np.asarray(x, dtype=f32))

    tf = c(inputs["track_features"])
    cf = c(inputs["current_features"])
    tb = c(inputs["track_boxes"])
    cb = c(inputs["current_boxes"])
    tt = c(inputs["track_time"]).reshape(-1, 1)
    ct = c(inputs["current_time"]).reshape(-1, 1)

    shared = {
        "cfT": c(cf.T),
        "curf": cf,
        "curg": c(np.concatenate([cb, ct], axis=1)),
        "wlinT": c(inputs["w_lin"].T),
        "wein1T": c(inputs["w_ein1"].T),
        "wein2T": c(inputs["w_ein2"].T),
        "we1T": c(inputs["w_e1"].T),
        "we2T": c(inputs["w_e2"].T),
        "wn1T": c(inputs["w_n1"].T),
        "wn2T": c(inputs["w_n2"].T),
        "wc1T": c(inputs["w_c1"].T),
        "wc2p8": c(np.tile(inputs["w_c2"].T, (1, 8))),
        "ballT": c(np.stack([
            inputs["b_lin"], inputs["b_ein1"], inputs["b_ein2"],
            inputs["b_e1"], inputs["b_e2"], inputs["b_n1"], inputs["b_n2"],
            inputs["b_c1"],
            np.broadcast_to(np.asarray(inputs["b_c2"], f32).reshape(1), (D,)),
        ], axis=1)),
    }
    in_maps = []
    for core in range(8):
        rows = slice(AL * core, AL * (core + 1))
        m = dict(shared)
        m["tfT"] = c(tf[rows].T)
        m["trkf"] = c(tf[rows])
        m["trkg"] = c(np.concatenate([tb[rows], tt[rows]], axis=1))
        in_maps.append(m)
    return in_maps


def run(trace=False, trace_cores=None, **inputs):
    from concourse.bass_utils import run_bass_kernel_spmd

    if trace:
        _install_ntff_hook()
    nc = _build()
    in_maps = _make_in_maps(inputs)
    res = run_bass_kernel_spmd(nc, in_maps, core_ids=list(range(8)),
                               trace=trace, trace_cores=trace_cores)
    full = np.empty((STEPS, A, B), np.float32)
    for core in range(8):
        full[:, AL * core:AL * (core + 1), :] = \
            res.results[core]["out"].reshape(STEPS, AL, B)
    return full, res


def kernel(**inputs):
    full, _ = run(trace=False, **inputs)
    return full


def _install_ntff_hook():
    import sys
    import types
    try:
        from antenv.axon_hooks import get_axon_ntff_profile_hook  # noqa: F401
        return
    except ImportError:
        pass
    import antenv
    from trn_agent_boot.trn_boot import _ntff_profile_via_ctypes

    mod = types.ModuleType("antenv.axon_hooks")
    holder = [_ntff_profile_via_ctypes("/opt/axon/libaxon_pjrt.so")]
    mod.get_axon_ntff_profile_hook = lambda: holder[0]
    mod.set_axon_ntff_profile_hook = lambda h: holder.__setitem__(0, h)
    sys.modules["antenv.axon_hooks"] = mod
    antenv.axon_hooks = mod


# revision 58
# speedup vs baseline: 1.1898x; 1.1898x over previous
"""AssignmentSimilarityNet GNN message-passing kernel for 8 Trainium2
NeuronCores.

Sharding: track (A) dimension split across 8 cores (32 tracks each).
Edge tensors, track embeds and messages-to-A stay local; messages-to-B
(sum over A) are all-reduced each step; MLP weights replicated.

Layouts (per core, feature-major: features on SBUF partitions):
  - edge/pair tensors: (128 feat, 8192 pairs), pairs a-major
    (pair = a*256 + b); compute in 512-pair groups (2 tracks) so every
    f32r matmul streams the max 512 columns per weight load.
  - the track-embed term of the edge MLP is constant along b, so it is
    folded into the h1-relu bias (beta) instead of a 512-cycle matmul.
  - msg_a rides on the u-relu: even groups via scalar-engine accum_out,
    odd groups via DVE tensor_reduce in phase C (DVE tensor_scalar's
    accum_out silently drops the max stage - do not use it).
  - msg_b lives entirely on the Pool(Q7) queue, one accumulator per
    group-half, pre-folded one group early; the last group's u-relu
    (on the fast Act queue) gates only two Q7 adds before the AR
    doorbell.  Each half gets its own AllReduce (AR is linear), so the
    first hides under the rest of the sweep.  AR results return via
    sync-DMA + DVE rounding copy so the Pool queue never blocks.
  - logits: c2 uses wc2 replicated into an 8-wide stationary; the copy
    takes PSUM row 0 into a (1, 8192) row, streamed out in 4 quarter
    DMAs (contiguous, no scatter).
  - per-step schedule: edge MLP sweep -> AR triggers -> classifier +
    track update + next-step group opens (AR cover) -> AR consume
    (current-embed update). Step 7 skips messages/AR/node updates
    entirely (dead code in the reference).
"""
import numpy as np

A = 256          # tracks
B = 256          # current detections
AL = A // 8      # tracks per core (32)
REID = 512
D = 128          # ND == ED
STEPS = 8
NP = AL * B      # pairs per core (8192)
CH = B           # half-group = one track row (256 pairs)
NCH = NP // CH   # 32
GCH = 2 * CH     # pair group = two track rows (512 pairs)
NG = NP // GCH   # 16
PREG = 4         # groups pre-opened for the next step during AR cover

_BUILD_CACHE = {}


def _build():
    if "nc" in _BUILD_CACHE:
        return _BUILD_CACHE["nc"]
    import concourse.bacc as bacc
    import concourse.mybir as mybir
    import concourse.tile as tile

    F32 = mybir.dt.float32
    F32R = mybir.dt.float32r
    F16 = mybir.dt.float16
    AF = mybir.ActivationFunctionType
    ALU = mybir.AluOpType

    nc = bacc.Bacc(None, target_bir_lowering=False)

    def din(name, shape):
        return nc.dram_tensor(name, shape, F32, kind="ExternalInput")

    tfT = din("tfT", [REID, AL])
    trkf = din("trkf", [AL, REID])
    cfT = din("cfT", [REID, B])
    curf = din("curf", [B, REID])
    trkg = din("trkg", [AL, 5])
    curg = din("curg", [B, 5])
    wlinT = din("wlinT", [REID, D])
    wein1T = din("wein1T", [6, D])
    wein2T = din("wein2T", [D, D])
    we1T = din("we1T", [4 * D, D])
    we2T = din("we2T", [D, D])
    wn1T = din("wn1T", [2 * D, D])
    wn2T = din("wn2T", [D, D])
    wc1T = din("wc1T", [D, D])
    wc2p8 = din("wc2p8", [D, 8])
    ballT = din("ballT", [D, 9])
    out = nc.dram_tensor("out", [STEPS, NP], F32, kind="ExternalOutput")

    with tile.TileContext(nc) as tc:
        with (
            tc.tile_pool(name="const", bufs=1) as cp,
            tc.tile_pool(name="state", bufs=1) as st,
            tc.tile_pool(name="work", bufs=1) as wk,
            tc.tile_pool(name="p1", bufs=5, space="PSUM") as pp1,
            tc.tile_pool(name="pmm", bufs=2, space="PSUM") as pmm,
            tc.tile_pool(name="plg", bufs=1, space="PSUM") as plgp,
            tc.tile_pool(name="dram", bufs=1, space="DRAM") as dr,
        ):
            # ---------------- collective warm-up ----------------
            # the first ncfw collective pays a 10-50us cold start; burn it
            # on a throwaway 4-byte AllReduce that runs concurrently with
            # the setup ladder (TOPSP/SDMA are separate silicon).
            warm_in = dr.tile([128, 1], F32, name="warm_in")
            warm_out = dr.tile([128, 1], F32, name="warm_out",
                               addr_space="Shared")
            nc.sync.dma_start(warm_in[:], ballT[:, 0:1])
            nc.gpsimd.collective_compute(
                "AllReduce", mybir.AluOpType.add,
                replica_groups=[list(range(8))],
                ins=[warm_in.opt()], outs=[warm_out.opt()])

            # ---------------- feature loads ----------------
            tf_t = st.tile([128, 4 * AL], F32R)       # 4 K-tiles of (128, 32)
            cf_t = st.tile([128, 4 * B], F32R)        # 4 K-tiles of (128, 256)
            nc.gpsimd.dma_start(
                tf_t[:, :].rearrange("p (j c) -> p j c", j=4),
                tfT.rearrange("(j p) c -> p j c", p=128))
            nc.gpsimd.dma_start(
                cf_t[:, :].rearrange("p (j c) -> p j c", j=4),
                cfT.rearrange("(j p) c -> p j c", p=128))
            trkf_t = wk.tile([AL, REID], F32)
            nc.sync.dma_start(trkf_t[:], trkf[:])
            curf_t0 = wk.tile([128, REID], F32)
            curf_t1 = wk.tile([128, REID], F32)
            nc.sync.dma_start(curf_t0[:], curf[0:128, :])
            nc.sync.dma_start(curf_t1[:], curf[128:256, :])
            trkg_t = wk.tile([AL, 5], F32)
            nc.sync.dma_start(trkg_t[:], trkg[:])
            curg_t0 = wk.tile([128, 5], F32)
            curg_t1 = wk.tile([128, 5], F32)
            nc.sync.dma_start(curg_t0[:], curg[0:128, :])
            nc.sync.dma_start(curg_t1[:], curg[128:256, :])

            # ---------------- weight / bias loads ----------------
            we1_t = cp.tile([128, 4 * D], F32R)
            wlin_t = cp.tile([128, 4 * D], F32R)
            wn1_t = cp.tile([128, 2 * D], F32R)
            nc.gpsimd.dma_start(
                we1_t[:, :].rearrange("p (j c) -> p j c", j=4),
                we1T.rearrange("(j p) c -> p j c", p=128))
            nc.gpsimd.dma_start(
                wlin_t[:, :].rearrange("p (j c) -> p j c", j=4),
                wlinT.rearrange("(j p) c -> p j c", p=128))
            nc.gpsimd.dma_start(
                wn1_t[:, :].rearrange("p (j c) -> p j c", j=2),
                wn1T.rearrange("(j p) c -> p j c", p=128))
            wein1_t = cp.tile([6, D], F32R)
            wein2_t = cp.tile([128, D], F32R)
            we2_t = cp.tile([128, D], F32R)
            wn2_t = cp.tile([128, D], F32R)
            wc1_t = cp.tile([128, D], F32R)
            wc2p8_t = cp.tile([128, 8], F32R)
            for dst, src in [(wein1_t, wein1T), (wein2_t, wein2T),
                             (we2_t, we2T), (wn2_t, wn2T), (wc1_t, wc1T),
                             (wc2p8_t, wc2p8)]:
                nc.gpsimd.dma_start(dst[:], src[:])
            bnames = ["blin", "bein1", "bein2", "be1", "be2", "bn1",
                      "bn2", "bc1", "bc2"]
            ball_t = cp.tile([128, 9], F32)
            nc.sync.dma_start(ball_t[:], ballT[:, :])
            biases = {nm: ball_t[:, i:i + 1] for i, nm in enumerate(bnames)}

            # ---------------- reid norms ----------------
            sq_t = wk.tile([AL, REID], F32, tag="sq", bufs=2)
            nc.vector.tensor_mul(sq_t[:], trkf_t[:], trkf_t[:])
            sst = wk.tile([AL, 1], F32)
            nc.vector.tensor_reduce(sst[:], sq_t[:], mybir.AxisListType.X, ALU.add)
            rt = wk.tile([AL, 1], F32)
            nc.vector.reciprocal(rt[:], sst[:])
            inv_t = wk.tile([AL, 1], F32)
            nc.scalar.activation(inv_t[:], rt[:], AF.Sqrt)

            invc = []
            for i, ct in enumerate((curf_t0, curf_t1)):
                sq_c = wk.tile([128, REID], F32, name=f"sq_c{i}", tag="sq", bufs=2)
                nc.vector.tensor_mul(sq_c[:], ct[:], ct[:])
                ssc = wk.tile([128, 1], F32, name=f"ssc{i}")
                nc.vector.tensor_reduce(ssc[:], sq_c[:], mybir.AxisListType.X,
                                        ALU.add)
                rc = wk.tile([128, 1], F32, name=f"rc{i}")
                nc.vector.reciprocal(rc[:], ssc[:])
                ic = wk.tile([128, 1], F32, name=f"ic{i}")
                nc.scalar.activation(ic[:], rc[:], AF.Sqrt)
                invc.append(ic)

            # ---------------- current-side geometry -> bcast rows ----------
            # rows of cstage: 0 xb, 1 yb, 2 hb, 3 ln hb, 4 ln wb, 5 tb, 6 invc
            cstage = dr.tile([7, B], F32)
            for i, gt in enumerate((curg_t0, curg_t1)):
                half = slice(128 * i, 128 * (i + 1))
                cg = wk.tile([128, 7], F32, name=f"cg{i}")
                nc.vector.tensor_add(cg[:, 0:1], gt[:, 0:1], gt[:, 2:3])
                nc.vector.tensor_scalar_mul(cg[:, 0:1], cg[:, 0:1], 0.5)
                nc.vector.tensor_add(cg[:, 1:2], gt[:, 1:2], gt[:, 3:4])
                nc.vector.tensor_scalar_mul(cg[:, 1:2], cg[:, 1:2], 0.5)
                nc.vector.tensor_sub(cg[:, 2:3], gt[:, 3:4], gt[:, 1:2])
                wb = wk.tile([128, 1], F32, name=f"wb{i}")
                nc.vector.tensor_sub(wb[:], gt[:, 2:3], gt[:, 0:1])
                nc.scalar.activation(cg[:, 3:4], cg[:, 2:3], AF.Ln)
                nc.scalar.activation(cg[:, 4:5], wb[:], AF.Ln)
                nc.vector.tensor_copy(cg[:, 5:6], gt[:, 4:5])
                nc.vector.tensor_copy(cg[:, 6:7], invc[i][:])
                nc.scalar.dma_start(cstage[:, half].transpose((1, 0)), cg[:])
            bcall = wk.tile([AL, 7 * B], F32)
            nc.scalar.dma_start(
                bcall[:], cstage[:, :].partition_broadcast(AL)
                .rearrange("p r b -> p (r b)"))
            bc = {nm: bcall[:, B * r:B * (r + 1)]
                  for r, nm in enumerate(["xb", "yb", "hb", "lnhb",
                                          "lnwb", "tb", "invc"])}

            # ---------------- track-side geometry scalars ----------------
            xt = wk.tile([AL, 1], F32)
            nc.vector.tensor_add(xt[:], trkg_t[:, 0:1], trkg_t[:, 2:3])
            nc.vector.tensor_scalar_mul(xt[:], xt[:], 0.5)
            yt = wk.tile([AL, 1], F32)
            nc.vector.tensor_add(yt[:], trkg_t[:, 1:2], trkg_t[:, 3:4])
            nc.vector.tensor_scalar_mul(yt[:], yt[:], 0.5)
            ht = wk.tile([AL, 1], F32)
            nc.vector.tensor_sub(ht[:], trkg_t[:, 3:4], trkg_t[:, 1:2])
            wt = wk.tile([AL, 1], F32)
            nc.vector.tensor_sub(wt[:], trkg_t[:, 2:3], trkg_t[:, 0:1])
            lnht = wk.tile([AL, 1], F32)
            nc.scalar.activation(lnht[:], ht[:], AF.Ln)
            lnwt = wk.tile([AL, 1], F32)
            nc.scalar.activation(lnwt[:], wt[:], AF.Ln)

            # ---------------- edge features (AL, B) each ----------------
            den = wk.tile([AL, B], F32)
            nc.vector.tensor_scalar_add(den[:], bc["hb"][:], ht[:, 0:1])
            rden = wk.tile([AL, B], F32)
            nc.vector.reciprocal(rden[:], den[:])

            fall = wk.tile([AL, 6 * B], F32R, name="f_all")
            feats = []
            f0 = fall[:, 0 * B:1 * B]
            nc.vector.tensor_scalar(f0, bc["xb"][:], xt[:, 0:1], 2.0,
                                    ALU.subtract, ALU.mult)
            nc.vector.tensor_mul(f0, f0, rden[:])
            feats.append(f0)
            f1 = fall[:, 1 * B:2 * B]
            nc.vector.tensor_scalar(f1, bc["yb"][:], yt[:, 0:1], 2.0,
                                    ALU.subtract, ALU.mult)
            nc.vector.tensor_mul(f1, f1, rden[:])
            feats.append(f1)
            f2 = fall[:, 2 * B:3 * B]
            nc.vector.tensor_scalar(f2, bc["lnwb"][:], -1.0, lnwt[:, 0:1],
                                    ALU.mult, ALU.add)
            feats.append(f2)
            f3 = fall[:, 3 * B:4 * B]
            nc.vector.tensor_scalar(f3, bc["lnhb"][:], -1.0, lnht[:, 0:1],
                                    ALU.mult, ALU.add)
            feats.append(f3)
            f4 = fall[:, 4 * B:5 * B]
            nc.vector.tensor_scalar_sub(f4, bc["tb"][:], trkg_t[:, 4:5])
            feats.append(f4)

            # dist_reid = 1 - gram * inv_t * inv_c
            pg = pmm.tile([AL, B], F32, tag="pmm")
            for j in range(4):
                nc.tensor.matmul(pg[:], tf_t[:, AL * j:AL * (j + 1)],
                                 cf_t[:, B * j:B * (j + 1)],
                                 start=(j == 0), stop=(j == 3))
            f5 = fall[:, 5 * B:6 * B]
            nc.vector.tensor_scalar(f5, pg[:], inv_t[:, 0:1], None,
                                    ALU.mult)
            nc.vector.tensor_mul(f5, f5, bc["invc"][:])
            nc.vector.tensor_scalar(f5, f5, -1.0, 1.0, ALU.mult, ALU.add)
            feats.append(f5)

            # ---------------- transpose features -> efT (6, 8192) ----------
            ef_stage = dr.tile([6, NP], F32R)
            nc.sync.dma_start(
                ef_stage[:, :].rearrange("f (a b) -> a f b", a=AL),
                fall[:].rearrange("a (f b) -> a f b", f=6))
            upds = [st.tile([128, NP], F32R, name="updA"),
                    st.tile([128, NP], F32R, name="updB")]
            efT_t = upds[0][0:6, :]
            nc.sync.dma_start(efT_t, ef_stage[:])

            # ---------------- initial node embeds ----------------
            pt = pmm.tile([128, AL], F32, tag="pmm")
            for j in range(4):
                nc.tensor.matmul(pt[:], wlin_t[:, 128 * j:128 * (j + 1)],
                                 tf_t[:, AL * j:AL * (j + 1)],
                                 start=(j == 0), stop=(j == 3))
            te = [st.tile([128, AL], F32R, name="teA"),
                  st.tile([128, AL], F32R, name="teB")]
            nc.scalar.activation(te[0][:], pt[:], AF.Relu,
                                 bias=biases["blin"][:, 0:1])
            pc_ = pmm.tile([128, B], F32, tag="pmm")
            for j in range(4):
                nc.tensor.matmul(pc_[:], wlin_t[:, 128 * j:128 * (j + 1)],
                                 cf_t[:, B * j:B * (j + 1)],
                                 start=(j == 0), stop=(j == 3))
            ce = [st.tile([128, B], F32R, name="ceA"),
                  st.tile([128, B], F32R, name="ceB")]
            nc.scalar.activation(ce[0][:], pc_[:], AF.Relu,
                                 bias=biases["blin"][:, 0:1])

            # ---------------- fixed_edge = mlp2(edge_feats) ----------------
            fixedT = st.tile([128, NP], F32R)
            ein_pend = None
            for g in range(NG):
                sl = slice(GCH * g, GCH * (g + 1))
                p1 = pp1.tile([128, GCH], F32, tag="p1")
                nc.tensor.matmul(p1[:], wein1_t[:], efT_t[:, sl],
                                 start=True, stop=True)
                h = wk.tile([128, GCH], F32R, tag="h1", bufs=4)
                if g % 2 == 0:
                    nc.scalar.activation(h[:], p1[:], AF.Relu,
                                         bias=biases["bein1"][:, 0:1])
                else:
                    nc.vector.tensor_scalar(h[:], p1[:],
                                            biases["bein1"][:, 0:1], 0.0,
                                            ALU.add, ALU.max)
                if ein_pend is not None:
                    pg_, ph = ein_pend
                    slp = slice(GCH * pg_, GCH * (pg_ + 1))
                    p2 = pmm.tile([128, GCH], F32, tag="pmm")
                    nc.tensor.matmul(p2[:], wein2_t[:], ph[:],
                                     start=True, stop=True)
                    if pg_ % 2 == 0:
                        nc.vector.tensor_scalar(fixedT[:, slp], p2[:],
                                                biases["bein2"][:, 0:1], 0.0,
                                                ALU.add, ALU.max)
                    else:
                        nc.scalar.activation(fixedT[:, slp], p2[:], AF.Relu,
                                             bias=biases["bein2"][:, 0:1])
                ein_pend = (g, h)
            pg_, ph = ein_pend
            slp = slice(GCH * pg_, GCH * (pg_ + 1))
            p2 = pmm.tile([128, GCH], F32, tag="pmm")
            nc.tensor.matmul(p2[:], wein2_t[:], ph[:], start=True, stop=True)
            nc.scalar.activation(fixedT[:, slp], p2[:], AF.Relu,
                                 bias=biases["bein2"][:, 0:1])

            # ---------------- message-passing steps ----------------
            def open_group(g, k, u_prev):
                """fixed/upd contributions of the edge-MLP hidden layer.
                The track-embed term is constant along b, so it rides in the
                h1 relu bias (beta) instead of a 512-cycle matmul."""
                sl = slice(GCH * g, GCH * (g + 1))
                p1 = pp1.tile([128, GCH], F32, tag="p1")
                if k == 0:
                    nc.tensor.matmul(p1[:], wefu_t[:], fixedT[:, sl],
                                     start=True, stop=False)
                else:
                    nc.tensor.matmul(p1[:], we1_t[:, 384:512], fixedT[:, sl],
                                     start=True, stop=False)
                    nc.tensor.matmul(p1[:], we1_t[:, 256:384], u_prev[:, sl],
                                     start=False, stop=False)
                return p1

            def make_beta(te_t, kk):
                # beta[:, a] = we1a @ te[:, a] + be1  (per-track h1 bias)
                ptc = pmm.tile([128, AL], F32, tag="pmm")
                nc.tensor.matmul(ptc[:], we1_t[:, 0:128], te_t[:],
                                 start=True, stop=True)
                bt = wk.tile([128, AL], F32, name=f"beta{kk}",
                             tag="beta", bufs=2)
                nc.vector.tensor_scalar_add(bt[:], ptc[:],
                                            biases["be1"][:, 0:1])
                return bt

            # step-0 edge MLP reads fixedT for both the upd and fixed slots
            wefu_t = cp.tile([128, D], F32R)
            nc.vector.tensor_add(wefu_t[:], we1_t[:, 256:384],
                                 we1_t[:, 384:512])

            preopened = {}
            beta = make_beta(te[0], 0)
            for g in range(PREG):
                preopened[g] = open_group(g, 0, fixedT)

            for k in range(STEPS):
                u_prev = fixedT if k == 0 else upds[(k + 1) % 2]
                u_cur = upds[k % 2]
                te_cur, te_nxt = te[k % 2], te[(k + 1) % 2]
                ce_cur, ce_nxt = ce[k % 2], ce[(k + 1) % 2]
                last = (k == STEPS - 1)

                msga = wk.tile([128, AL], F32, tag="msga", bufs=2)
                # msg_b accumulators: two halves (groups 0-7 / 8-15), each
                # split DVE/GpSimd.  AllReduce is linear, so each half gets
                # its own AR: the first is triggered mid-sweep and hides
                # fully under the rest of phase A.
                macc = {}
                mb_outs = []

                def macc_tile(key):
                    if key not in macc:
                        macc[key] = wk.tile([128, GCH], F32,
                                            name=f"macc_{key}_{k}",
                                            tag=f"macc{key}", bufs=1)
                        return macc[key], True
                    return macc[key], False

                prefold = {}

                def make_prefold(half):
                    # the accumulator is complete one group before the half
                    # ends: collapse its two a-parity halves early so the
                    # last group's u relu gates only two Q7 adds before the
                    # AR doorbell.  The whole msg_b path lives on the Pool
                    # queue, which tracks phase A closely (DVE/Act are the
                    # backlogged queues).
                    pf = wk.tile([128, B], F32, name=f"pf{half}_{k}",
                                 tag="pfold", bufs=2)
                    qa, _ = macc_tile(f"q{half}")
                    nc.gpsimd.tensor_add(pf[:], qa[:, 0:B], qa[:, B:GCH])
                    prefold[half] = pf

                def trigger_ar(half, u_f):
                    pf = prefold.pop(half)
                    msgb = wk.tile([128, B], F32, tag="msgb", bufs=2)
                    nc.gpsimd.tensor_add(msgb[:], pf[:], u_f[:, 0:B])
                    nc.gpsimd.tensor_add(msgb[:], msgb[:], u_f[:, B:GCH])
                    mb_in = dr.tile([128, B], F32, tag="mbin", bufs=4)
                    mb_out = dr.tile([128, B], F32, tag="mbout", bufs=4,
                                     addr_space="Shared")
                    nc.sync.dma_start(mb_in[:], msgb[:])
                    nc.gpsimd.collective_compute(
                        "AllReduce", mybir.AluOpType.add,
                        replica_groups=[list(range(8))],
                        ins=[mb_in.opt()], outs=[mb_out.opt()])
                    mb_outs.append(mb_out)

                # ---- phase A: edge MLP sweep (16 groups of 2 tracks) ----
                def tail_a(g):
                    sl = slice(GCH * g, GCH * (g + 1))
                    h = h1s.pop(g)
                    p2 = pmm.tile([128, GCH], F32, tag="pmm")
                    nc.tensor.matmul(p2[:], we2_t[:], h[:],
                                     start=True, stop=True)
                    if last:
                        nc.scalar.activation(u_cur[:, sl], p2[:], AF.Relu,
                                             bias=biases["be2"][:, 0:1])
                        return
                    half = g // (NG // 2)
                    pos = g % (NG // 2)
                    if g % 2 == 0 or pos == NG // 2 - 1:
                        # Act relu halves; accum_out gives msg_a for free.
                        # Also used for the last group of each half so the
                        # AR trigger chain rides the fast Act queue.
                        for hh in range(2):
                            hsl = slice(GCH * g + CH * hh,
                                        GCH * g + CH * (hh + 1))
                            nc.scalar.activation(
                                u_cur[:, hsl], p2[:, CH * hh:CH * (hh + 1)],
                                AF.Relu, bias=biases["be2"][:, 0:1],
                                accum_out=msga[:, 2 * g + hh:2 * g + hh + 1])
                    else:
                        # plain DVE relu; msg_a for these groups comes from
                        # tensor_reduce during phase C
                        nc.vector.tensor_scalar(u_cur[:, sl], p2[:],
                                                biases["be2"][:, 0:1], 0.0,
                                                ALU.add, ALU.max)
                    u_f = u_cur[:, sl].bitcast(F32)
                    if pos == NG // 2 - 1:
                        # last group of the half feeds the AR input directly
                        trigger_ar(half, u_f)
                        return
                    acc, first = macc_tile(f"q{half}")
                    if first:
                        nc.gpsimd.tensor_copy(acc[:], u_f)
                    else:
                        nc.gpsimd.tensor_add(acc[:], acc[:], u_f)
                    if pos == NG // 2 - 2:
                        make_prefold(half)

                h1s = {}
                for g in range(NG):
                    if g in preopened:
                        p1 = preopened.pop(g)
                    else:
                        p1 = open_group(g, k, u_prev)
                    nc.tensor.matmul(
                        p1[:], we1_t[:, 128:256],
                        ce_cur[:, :].to_broadcast((128, B, 2))
                        .transpose((0, 2, 1)), start=False, stop=True)
                    h = wk.tile([128, GCH], F32R, tag="h1", bufs=4)
                    for hh in range(2):
                        a = 2 * g + hh
                        csl = slice(CH * hh, CH * (hh + 1))
                        if g % 2 == 1:
                            nc.scalar.activation(h[:, csl], p1[:, csl],
                                                 AF.Relu,
                                                 bias=beta[:, a:a + 1])
                        else:
                            nc.vector.tensor_scalar(h[:, csl], p1[:, csl],
                                                    beta[:, a:a + 1], 0.0,
                                                    ALU.add, ALU.max)
                    h1s[g] = h
                    if g > 2:
                        tail_a(g - 3)
                tail_a(NG - 3)
                tail_a(NG - 2)
                tail_a(NG - 1)

                # ---- phase C: classifier for this step (AR cover) ----
                # c2 with wc2 replicated into an 8-wide stationary: every
                # PSUM row 0..7 gets the group's logits; the copy takes row
                # g%8, so lg_sb rows interleave groups and the out DMA is 8
                # partitions x 4KB, all f32r precision, no transpose.
                lg_sb = wk.tile([1, NP], F32, tag="lgs", bufs=1)
                hcs = {}

                def c2_mm(g):
                    hc = hcs.pop(g)
                    pl2 = plgp.tile([8, GCH], F32, tag="plg")
                    nc.tensor.matmul(pl2[:], wc2p8_t[:], hc[:],
                                     start=True, stop=True)
                    dst = lg_sb[0:1, GCH * g:GCH * (g + 1)]
                    nc.scalar.activation(dst, pl2[0:1, :], AF.Identity,
                                         bias=biases["bc2"][0:1, 0:1])
                    if g % 4 == 3:   # stream the finished quarter out
                        q4 = g // 4
                        nc.sync.dma_start(
                            out[k:k + 1, 2048 * q4:2048 * (q4 + 1)],
                            lg_sb[0:1, 2048 * q4:2048 * (q4 + 1)])

                for g in range(NG):
                    sl = slice(GCH * g, GCH * (g + 1))
                    p3 = pmm.tile([128, GCH], F32, tag="pmm")
                    nc.tensor.matmul(p3[:], wc1_t[:], u_cur[:, sl],
                                     start=True, stop=True)
                    hc = wk.tile([128, GCH], F32R, tag="hc", bufs=3)
                    nc.vector.tensor_scalar(hc[:], p3[:],
                                            biases["bc1"][:, 0:1], 0.0,
                                            ALU.add, ALU.max)
                    hcs[g] = hc
                    if g > 1:
                        c2_mm(g - 2)
                c2_mm(NG - 2)
                c2_mm(NG - 1)

                if last:
                    break

                # ---- track-side node update (local msg_a) ----
                for g in (1, 3, 5, 9, 11, 13):   # odd groups' msg_a via reduce
                    sl = slice(GCH * g, GCH * (g + 1))
                    nc.vector.tensor_reduce(
                        msga[:, 2 * g:2 * g + 2],
                        u_cur[:, sl].bitcast(F32)
                        .rearrange("p (a b) -> p a b", a=2),
                        mybir.AxisListType.X, ALU.add)
                msga_r = wk.tile([128, AL], F32R, tag="msgar", bufs=2)
                nc.vector.tensor_copy(msga_r[:], msga[:])
                pt1 = pmm.tile([128, AL], F32, tag="pmm")
                nc.tensor.matmul(pt1[:], wn1_t[:, 0:128], te_cur[:],
                                 start=True, stop=False)
                nc.tensor.matmul(pt1[:], wn1_t[:, 128:256],
                                 msga_r[:], start=False, stop=True)
                tn1 = wk.tile([128, AL], F32R, tag="tn1", bufs=2)
                nc.vector.tensor_scalar(tn1[:], pt1[:], biases["bn1"][:, 0:1],
                                        0.0, ALU.add, ALU.max)
                pt2 = pmm.tile([128, AL], F32, tag="pmm")
                nc.tensor.matmul(pt2[:], wn2_t[:], tn1[:], start=True,
                                 stop=True)
                nc.vector.tensor_scalar(te_nxt[:], pt2[:], biases["bn2"][:, 0:1],
                                        0.0, ALU.add, ALU.max)

                # ---- pre-open next step's first groups (more AR cover) ----
                beta = make_beta(te_nxt, k + 1)
                for g in range(PREG):
                    preopened[g] = open_group(g, k + 1, u_cur)

                # ---- phase D: consume ARs -> current-embed update ----
                # AR results land via sync-engine DMAs, then a DVE copy
                # rounds f32 -> f32r for the matmul.
                msgb_fs = []
                for hf, mo in enumerate(mb_outs):
                    ms = wk.tile([128, B], F32, name=f"mbs{hf}_{k}",
                                 tag="msgbs", bufs=4)
                    nc.sync.dma_start(ms[:], mo[:])
                    mf = wk.tile([128, B], F32R, name=f"mbf{hf}_{k}",
                                 tag="msgbf", bufs=4)
                    nc.vector.tensor_copy(mf[:], ms[:])
                    msgb_fs.append(mf)
                pc1 = pmm.tile([128, B], F32, tag="pmm")
                nc.tensor.matmul(pc1[:], wn1_t[:, 0:128], ce_cur[:],
                                 start=True, stop=False)
                nc.tensor.matmul(pc1[:], wn1_t[:, 128:256], msgb_fs[0][:],
                                 start=False, stop=False)
                nc.tensor.matmul(pc1[:], wn1_t[:, 128:256], msgb_fs[1][:],
                                 start=False, stop=True)
                cn1 = wk.tile([128, B], F32R, tag="cn1", bufs=2)
                nc.scalar.activation(cn1[:], pc1[:], AF.Relu,
                                     bias=biases["bn1"][:, 0:1])
                pc2 = pmm.tile([128, B], F32, tag="pmm")
                nc.tensor.matmul(pc2[:], wn2_t[:], cn1[:], start=True,
                                 stop=True)
                nc.scalar.activation(ce_nxt[:], pc2[:], AF.Relu,
                                     bias=biases["bn2"][:, 0:1])

    nc.finalize()
    _BUILD_CACHE["nc"] = nc
    return nc


def _make_in_maps(inputs):
    f32 = np.float32

    def c(x):
        return np.ascontiguousarray(np.asarray(x, dtype=f32))

    tf = c(inputs["track_features"])
    cf = c(inputs["current_features"])
    tb = c(inputs["track_boxes"])
    cb = c(inputs["current_boxes"])
    tt = c(inputs["track_time"]).reshape(-1, 1)
    ct = c(inputs["current_time"]).reshape(-1, 1)

    shared = {
        "cfT": c(cf.T),
        "curf": cf,
        "curg": c(np.concatenate([cb, ct], axis=1)),
        "wlinT": c(inputs["w_lin"].T),
        "wein1T": c(inputs["w_ein1"].T),
        "wein2T": c(inputs["w_ein2"].T),
        "we1T": c(inputs["w_e1"].T),
        "we2T": c(inputs["w_e2"].T),
        "wn1T": c(inputs["w_n1"].T),
        "wn2T": c(inputs["w_n2"].T),
        "wc1T": c(inputs["w_c1"].T),
        "wc2p8": c(np.tile(inputs["w_c2"].T, (1, 8))),
        "ballT": c(np.stack([
            inputs["b_lin"], inputs["b_ein1"], inputs["b_ein2"],
            inputs["b_e1"], inputs["b_e2"], inputs["b_n1"], inputs["b_n2"],
            inputs["b_c1"],
            np.broadcast_to(np.asarray(inputs["b_c2"], f32).reshape(1), (D,)),
        ], axis=1)),
    }
    in_maps = []
    for core in range(8):
        rows = slice(AL * core, AL * (core + 1))
        m = dict(shared)
        m["tfT"] = c(tf[rows].T)
        m["trkf"] = c(tf[rows])
        m["trkg"] = c(np.concatenate([tb[rows], tt[rows]], axis=1))
        in_maps.append(m)
    return in_maps


def run(trace=False, trace_cores=None, **inputs):
    from concourse.bass_utils import run_bass_kernel_spmd

    if trace:
        _install_ntff_hook()
    nc = _build()
    in_maps = _make_in_maps(inputs)
    res = run_bass_kernel_spmd(nc, in_maps, core_ids=list(range(8)),
                               trace=trace, trace_cores=trace_cores)
    full = np.empty((STEPS, A, B), np.float32)
    for core in range(8):
        full[:, AL * core:AL * (core + 1), :] = \
            res.results[core]["out"].reshape(STEPS, AL, B)
    return full, res


def kernel(**inputs):
    full, _ = run(trace=False, **inputs)
    return full


def _install_ntff_hook():
    import sys
    import types
    try:
        from antenv.axon_hooks import get_axon_ntff_profile_hook  # noqa: F401
        return
    except ImportError:
        pass
    import antenv
    from trn_agent_boot.trn_boot import _ntff_profile_via_ctypes

    mod = types.ModuleType("antenv.axon_hooks")
    holder = [_ntff_profile_via_ctypes("/opt/axon/libaxon_pjrt.so")]
    mod.get_axon_ntff_profile_hook = lambda: holder[0]
    mod.set_axon_ntff_profile_hook = lambda h: holder.__setitem__(0, h)
    sys.modules["antenv.axon_hooks"] = mod
    antenv.axon_hooks = mod
